# revision 14
# baseline (speedup 1.0000x reference)
"""Trainium2 Bass kernel for Llama GQA attention (no mask), 8-way tensor
parallel over KV heads.

Problem shapes (hardcoded):
  x  (2, 2048, 4096) f32
  wq (4096, 4096), wk (1024, 4096), wv (1024, 4096), wo (4096, 4096) f32
  NUM_HEADS=32, NUM_KV_HEADS=8, HEAD_DIM=128, GQA group g=4

Sharding: core c owns KV head c (4 Q heads). x replicated (pre-transposed
to xT on host), wq/wk/wv sharded on output dim (pre-transposed host-side),
wo sharded on input dim. Each core computes a partial (4096, 4096) output
(its heads' contribution through wo, bf16); host upcasts and sums the 8
partials.

All data bf16 (FWL weight loads, halved DMA), PSUM accumulation f32.

v9 structure - one static 8-bank PSUM layout shared by every phase
(s_ps [128,1024]x2 = 4 banks, pv_ps [128,512]x2, o_ps [128,512]x2),
which lets batch-1 projection work overlap batch-0 attention:

  P1a   full projections for batch 0 (q heads in s_ps bank pairs,
        k/v in pv_ps, v-transposes through o_ps)
  P1b   q-projections only for batch 1
  OVLP  attention chunks of batch 0 (scores/PV only; outproj and the
        softmax normalization deferred) with batch-1 k/v projection
        matmuls interleaved 2-per-slot as PE filler in o_ps - the
        attention-only loop would otherwise be exp(ACT)-bound
  SEAM  batch-1 v transposes (o_ps)
  FINAL attention chunks of batch 1 + deferred normalizations + ALL
        eight chunks' output-projection groups drained 2 per slot
        from a ready-queue (o_ps shared by den-replicate MMs and
        outproj groups)

Other key scheduling: PV matmuls run two slots behind their exp (PE
never waits on ACT); softmax denominators are summed on the DVE and
partition-reduced/broadcast by one ones-matmul per head; PSUM
evacuations alternate DVE/ACT; bulk weight DMAs are batched (gpsimd
SWDGE costs ~0.6us serial Q7 time per dma_start).
"""

import sys
from contextlib import ExitStack

import numpy as np

sys.path.insert(0, "/opt/trn_rl_repo")

import concourse.bass as bass  # noqa: E402
import concourse.tile as tile  # noqa: E402
from concourse import bacc, mybir  # noqa: E402
from concourse.bass_utils import run_bass_kernel_spmd  # noqa: E402
from concourse.masks import make_identity  # noqa: E402

NCORES = 8
B, S, H = 2, 2048, 4096
T = B * S                      # 4096 flattened tokens
D = 128                        # head dim
G = 4                          # q heads per core (GQA group)
HK = 32                        # h k-tiles (4096 / 128)
TT = T // 128                  # 32 token tiles
NJ = T // 512                  # 8 token chunks of 512
SJ = S // 512                  # 4 tq chunks per batch
SI = S // 128                  # 16 tk tiles per batch
SCALE = float(1.0 / np.sqrt(D))

F32 = mybir.dt.float32
BF16 = mybir.dt.bfloat16
COPY = mybir.ActivationFunctionType.Copy
EXP = mybir.ActivationFunctionType.Exp


def build_nc():
    nc = bacc.Bacc("TRN2", target_bir_lowering=False, debug=False,
                   enable_asserts=True, num_devices=NCORES)
    xt = nc.declare_dram_parameter("xt", [H, T], BF16, isOutput=False)
    wqt = nc.declare_dram_parameter("wqt", [H, G * D], BF16, isOutput=False)
    wkt = nc.declare_dram_parameter("wkt", [H, D], BF16, isOutput=False)
    wvt = nc.declare_dram_parameter("wvt", [H, D], BF16, isOutput=False)
    wot = nc.declare_dram_parameter("wot", [G * D, H], BF16, isOutput=False)
    ones = nc.declare_dram_parameter("ones", [128, 128], BF16, isOutput=False)
    out = nc.declare_dram_parameter("out", [T, H], BF16, isOutput=True)

    xt_r = xt.ap().rearrange("(k p) t -> p k t", p=128)     # [128, 32, T]
    wqt_r = wqt.ap().rearrange("(k p) m -> p k m", p=128)   # [128, 32, 512]
    wkt_r = wkt.ap().rearrange("(k p) m -> p k m", p=128)   # [128, 32, 128]
    wvt_r = wvt.ap().rearrange("(k p) m -> p k m", p=128)   # [128, 32, 128]
    wot_r = wot.ap().rearrange("(k p) n -> p k n", p=128)   # [128, 4, T]
    out_r = out.ap()

    with tile.TileContext(nc) as tc:
        with ExitStack() as ctx:
            persist = ctx.enter_context(tc.tile_pool(name="persist", bufs=1))
            q_sb = persist.tile([128, G, T], BF16)       # qT per head, 4MB
            k_sb = persist.tile([128, T], BF16)          # kT, 1MB
            v_sb = persist.tile([128, TT, D], BF16)      # v natural, 1MB
            ones_sb = persist.tile([128, 128], BF16)
            ident = persist.tile([128, 128], F32)
            wo_sb = persist.tile([128, G, T], BF16)      # 4MB resident

            # unified PSUM layout (8 banks, whole kernel)
            psS = ctx.enter_context(tc.tile_pool(name="psS", bufs=2, space="PSUM"))
            psPV = ctx.enter_context(tc.tile_pool(name="psPV", bufs=2, space="PSUM"))
            psO = ctx.enter_context(tc.tile_pool(name="psO", bufs=2, space="PSUM"))

            # SBUF pools (static)
            xpool = ctx.enter_context(tc.tile_pool(name="xpool", bufs=6))
            vstg = ctx.enter_context(tc.tile_pool(name="vstg", bufs=2))
            vst1 = ctx.enter_context(tc.tile_pool(name="vst1", bufs=4))
            ppool = ctx.enter_context(tc.tile_pool(name="ppool", bufs=4))
            apool = ctx.enter_context(tc.tile_pool(name="apool", bufs=5))
            accpool = ctx.enter_context(tc.tile_pool(name="accpool", bufs=1))
            dpool = ctx.enter_context(tc.tile_pool(name="dpool", bufs=18))
            rpool = ctx.enter_context(tc.tile_pool(name="rpool", bufs=2))
            opool = ctx.enter_context(tc.tile_pool(name="opool", bufs=3))

            deferred = []    # (den1, a_t) stashed by the overlap
            oqueue = []      # ready outproj groups: (b, j, a_ch, grp)
            a_sets = {}      # bj index -> a_ch list
            vst1_tiles = []  # staged batch-1 v (awaiting seam transposes)
            prev_v = [None]

            # ---------------- attention machinery ----------------
            def pv_step(pd):
                # PV of an exp chunk two slots old + its den-add; on the
                # head's last chunk fold den and return finalize record
                pv_ps, p_t, bb, g, acc, den1, a_t = pd
                for h in range(2):
                    ti = bb * SI + 2 * g + h
                    st = g == 0 and h == 0
                    sp = g == SI // 2 - 1 and h == 1
                    nc.tensor.matmul(
                        pv_ps, v_sb[:, ti, :],
                        p_t[:, h * 512:(h + 1) * 512],
                        start=st, stop=sp)
                if g == 0:
                    nc.vector.tensor_copy(acc, p_t)
                else:
                    nc.vector.tensor_add(acc, acc, p_t)
                if g == SI // 2 - 1:
                    nc.vector.tensor_add(den1, acc[:, 0:512],
                                         acc[:, 512:1024])
                    return (pv_ps, den1, a_t)
                return None

            def finalize_raw(pend):
                # overlap path: stash unnormalized PV and keep den1
                pv_ps, den1, a_t = pend
                nc.vector.tensor_copy(a_t, pv_ps)
                deferred.append((den1, a_t))

            fin_count = [0, 0]   # [flush_deferred, finalize_now] counts

            def _enqueue_chunk(bj):
                for grp in range(32):
                    oqueue.append((bj // SJ, bj % SJ, a_sets[bj], grp))

            def finalize_now(pend):
                # final path: replicate den, reciprocal, normalize
                pv_ps, den1, a_t = pend
                den_ps = psO.tile([128, 512], F32, name="o_ps")
                nc.tensor.matmul(den_ps, ones_sb, den1, start=True,
                                 stop=True)
                rec_t = rpool.tile([128, 512], F32)
                nc.vector.reciprocal_approx_fast(out=rec_t, in_=den_ps)
                nc.vector.tensor_mul(a_t, pv_ps, rec_t)
                fin_count[1] += 1
                if fin_count[1] % 4 == 0:
                    _enqueue_chunk(SJ + fin_count[1] // 4 - 1)

            def flush_deferred():
                # normalize one stashed overlap head (a_t in place)
                den1, a_t = deferred.pop(0)
                den_ps = psO.tile([128, 512], F32, name="o_ps")
                nc.tensor.matmul(den_ps, ones_sb, den1, start=True,
                                 stop=True)
                rec_t = rpool.tile([128, 512], F32)
                nc.vector.reciprocal_approx_fast(out=rec_t, in_=den_ps)
                nc.vector.tensor_mul(a_t, a_t, rec_t)
                fin_count[0] += 1
                if fin_count[0] % 4 == 0:
                    _enqueue_chunk(fin_count[0] // 4 - 1)

            def outproj_group(pb, pj, pa, grp):
                tt2, n = grp // NJ, grp % NJ
                t0 = pb * S + pj * 512 + tt2 * 128
                o_ps = psO.tile([128, 512], F32, name="o_ps")
                for m in range(G):
                    nc.tensor.matmul(
                        o_ps, pa[m][:, tt2 * 128:(tt2 + 1) * 128],
                        wo_sb[:, m, n * 512:(n + 1) * 512],
                        start=(m == 0), stop=(m == G - 1))
                o_t = opool.tile([128, 512], BF16)
                if grp % 2 == 0:
                    nc.vector.tensor_copy(o_t, o_ps)
                else:
                    nc.scalar.activation(out=o_t, in_=o_ps, func=COPY)
                nc.sync.dma_start(
                    out=out_r[t0:t0 + 128, n * 512:(n + 1) * 512],
                    in_=o_t)

            def attn_chunk(bj, overlap_gen, pvq, pending, n_flush,
                           pop_groups):
                b, j = bj // SJ, bj % SJ
                tqsl = slice(b * S + j * 512, b * S + (j + 1) * 512)
                a_ch = [apool.tile([128, 512], BF16, name=f"a_ch{m}")
                        for m in range(G)]
                a_sets[bj] = a_ch
                for m in range(G):
                    pv_ps = psPV.tile([128, 512], F32, name="pv_ps")
                    acc = accpool.tile([128, 1024], F32)
                    den1 = dpool.tile([128, 512], BF16)
                    for g in range(SI // 2):
                        s_ps = psS.tile([128, 1024], F32, name="s_ps")
                        for h in range(2):
                            ti = b * SI + 2 * g + h
                            nc.tensor.matmul(
                                s_ps[:, h * 512:(h + 1) * 512],
                                k_sb[:, ti * 128:(ti + 1) * 128],
                                q_sb[:, m, tqsl], start=True, stop=True)
                        p_t = ppool.tile([128, 1024], BF16)
                        nc.scalar.activation(out=p_t, in_=s_ps, func=EXP,
                                             scale=SCALE)
                        # PE filler / deferred-normalize / outproj slots
                        if overlap_gen is not None:
                            for _ in range(2):
                                next(overlap_gen, None)
                        if n_flush[0] > 0 and deferred:
                            flush_deferred()
                            n_flush[0] -= 1
                        if pop_groups:
                            for _ in range(2):
                                if oqueue:
                                    gb, gj, ga, gg = oqueue.pop(0)
                                    outproj_group(gb, gj, ga, gg)
                        if len(pvq) >= 2:
                            fin = pv_step(pvq.pop(0))
                            if fin is not None:
                                pending[0] = fin
                        pvq.append((pv_ps, p_t, b, g, acc, den1, a_ch[m]))
                        if g == 3 and pending[0] is not None:
                            if pop_groups:
                                finalize_now(pending[0])
                            else:
                                finalize_raw(pending[0])
                            pending[0] = None
                return a_ch

            # ---------------- weight / const DMAs + phase 1 ----------------
            with ExitStack() as c1:
                wpool = c1.enter_context(tc.tile_pool(name="wpool", bufs=1))
                wq_t = wpool.tile([128, HK, G * D], BF16)   # 4MB
                wk_t = wpool.tile([128, HK, D], BF16)       # 1MB
                wv_t = wpool.tile([128, HK, D], BF16)       # 1MB
                # first k-tiles fine-split across two queues for a fast
                # head; the bulk batched (SWDGE Q7 cost ~0.6us per trigger)
                for k in range(4):
                    if k < 2:
                        for q8 in range(8):
                            eng = [nc.gpsimd, nc.scalar][q8 % 2]
                            eng.dma_start(
                                out=wq_t[:, k, q8 * 64:(q8 + 1) * 64],
                                in_=wqt_r[:, k, q8 * 64:(q8 + 1) * 64])
                        for q2 in range(2):
                            psl = slice(q2 * 64, (q2 + 1) * 64)
                            nc.gpsimd.dma_start(out=wk_t[psl, k, :],
                                                in_=wkt_r[psl, k, :])
                            nc.scalar.dma_start(out=wv_t[psl, k, :],
                                                in_=wvt_r[psl, k, :])
                    else:
                        nc.gpsimd.dma_start(out=wq_t[:, k, :], in_=wqt_r[:, k, :])
                        nc.gpsimd.dma_start(out=wk_t[:, k, :], in_=wkt_r[:, k, :])
                        nc.gpsimd.dma_start(out=wv_t[:, k, :], in_=wvt_r[:, k, :])
                for k0 in range(4, HK, 7):
                    k1 = min(k0 + 7, HK)
                    nc.gpsimd.dma_start(out=wq_t[:, k0:k1, :],
                                        in_=wqt_r[:, k0:k1, :])
                nc.gpsimd.dma_start(out=wk_t[:, 4:HK, :], in_=wkt_r[:, 4:HK, :])
                nc.gpsimd.dma_start(out=wv_t[:, 4:HK, :], in_=wvt_r[:, 4:HK, :])
                nc.gpsimd.dma_start(out=ones_sb, in_=ones.ap())
                for k in range(G):
                    nc.gpsimd.dma_start(out=wo_sb[:, k, :], in_=wot_r[:, k, :])
                make_identity(nc, ident)

                def v_transpose(pj, pv_st):
                    # f32 transposes through the shared o_ps bank
                    vt = psO.tile([128, 512], F32, name="o_ps")
                    for tt in range(4):
                        nc.tensor.transpose(
                            vt[:, tt * 128:(tt + 1) * 128],
                            pv_st[:, tt * 128:(tt + 1) * 128], ident)
                    nc.scalar.activation(
                        out=v_sb[:, 4 * pj:4 * pj + 4, :], in_=vt,
                        func=COPY)

                def p1_chunk(j, do_kv):
                    tsl = slice(j * 512, (j + 1) * 512)
                    q01 = psS.tile([128, 1024], F32, name="s_ps")
                    q23 = psS.tile([128, 1024], F32, name="s_ps")
                    if do_kv:
                        k_ps = psPV.tile([128, 512], F32, name="pv_ps")
                        v_ps = psPV.tile([128, 512], F32, name="pv_ps")
                    for k in range(HK):
                        if k == 0 and prev_v[0] is not None:
                            v_transpose(*prev_v[0])
                            prev_v[0] = None
                        x_t = xpool.tile([128, 512], BF16)
                        if j == 0 and k < 4:
                            for q4 in range(4):
                                psl = slice(q4 * 32, (q4 + 1) * 32)
                                nc.sync.dma_start(out=x_t[psl, :],
                                                  in_=xt_r[psl, k, tsl])
                        else:
                            nc.sync.dma_start(out=x_t, in_=xt_r[:, k, tsl])
                        st, sp = k == 0, k == HK - 1
                        for m in range(G):
                            tgt = (q01, q23)[m // 2]
                            nc.tensor.matmul(
                                tgt[:, (m % 2) * 512:(m % 2) * 512 + 512],
                                wq_t[:, k, m * D:(m + 1) * D], x_t,
                                start=st, stop=sp)
                        if do_kv:
                            nc.tensor.matmul(k_ps, wk_t[:, k, :], x_t,
                                             start=st, stop=sp)
                            nc.tensor.matmul(v_ps, wv_t[:, k, :], x_t,
                                             start=st, stop=sp)
                    # evacuations: q pairs via strided [128, 2, 512] APs
                    nc.scalar.activation(out=q_sb[:, 0:2, tsl], in_=q01,
                                         func=COPY)
                    nc.vector.tensor_copy(q_sb[:, 2:4, tsl], q23)
                    if do_kv:
                        nc.scalar.activation(out=k_sb[:, tsl], in_=k_ps,
                                             func=COPY)
                        v_st = vstg.tile([128, 512], F32)
                        nc.vector.tensor_copy(v_st, v_ps)
                        prev_v[0] = (j, v_st)

                # ---------------- P1a: batch 0, full ----------------
                for j in range(SJ):
                    p1_chunk(j, True)
                # ---------------- P1b: batch 1, q only ----------------
                for j in range(SJ, NJ):
                    p1_chunk(j, False)

                # ------- batch-1 k/v filler generator (overlap) -------
                def kv_filler():
                    for j in range(SJ, NJ):
                        tsl = slice(j * 512, (j + 1) * 512)
                        k_ps = psO.tile([128, 512], F32, name="o_ps")
                        v_ps = psO.tile([128, 512], F32, name="o_ps")
                        for k in range(HK):
                            x_t = xpool.tile([128, 512], BF16)
                            nc.sync.dma_start(out=x_t, in_=xt_r[:, k, tsl])
                            st, sp = k == 0, k == HK - 1
                            nc.tensor.matmul(k_ps, wk_t[:, k, :], x_t,
                                             start=st, stop=sp)
                            yield
                            nc.tensor.matmul(v_ps, wv_t[:, k, :], x_t,
                                             start=st, stop=sp)
                            if sp:
                                nc.scalar.activation(out=k_sb[:, tsl],
                                                     in_=k_ps, func=COPY)
                                v_st = vst1.tile([128, 512], F32)
                                nc.vector.tensor_copy(v_st, v_ps)
                                vst1_tiles.append((j, v_st))
                            yield

                # ---------------- OVERLAP: batch-0 attention ----------------
                gen = kv_filler()
                pvq, pending, nf = [], [None], [0]
                for bj in range(SJ):
                    attn_chunk(bj, gen, pvq, pending, nf, False)
                # drain overlap pipeline
                while next(gen, "END") != "END":
                    pass
                while pvq:
                    fin = pv_step(pvq.pop(0))
                    if fin is not None:
                        pending[0] = fin
                finalize_raw(pending[0])
                pending[0] = None

            # wpool closed: wq/wk/wv SBUF freed

            # ---------------- SEAM: batch-1 v transposes ----------------
            for pj, v_st in vst1_tiles:
                vt = psO.tile([128, 512], F32, name="o_ps")
                for tt in range(4):
                    nc.tensor.transpose(
                        vt[:, tt * 128:(tt + 1) * 128],
                        v_st[:, tt * 128:(tt + 1) * 128], ident)
                nc.scalar.activation(
                    out=v_sb[:, 4 * pj:4 * pj + 4, :], in_=vt,
                    func=COPY)

            # ---------------- FINAL: batch-1 attention ----------------
            # batch-0 outproj groups become poppable; their heads are
            # normalized by the first 16 per-slot flush_deferred calls
            # (chunk bj fully normalized by slot 4*bj+3; its first group
            # pops at the earliest at slot 6 - order is safe)
            pvq, pending, nf = [], [None], [16]
            for bj in range(SJ, 2 * SJ):
                attn_chunk(bj, None, pvq, pending, nf, True)
            # drain
            while pvq:
                fin = pv_step(pvq.pop(0))
                if fin is not None:
                    pending[0] = fin
            finalize_now(pending[0])
            while oqueue:
                gb, gj, ga, gg = oqueue.pop(0)
                outproj_group(gb, gj, ga, gg)
    nc.compile()
    return nc


_NC_CACHE = None


def _get_nc():
    global _NC_CACHE
    if _NC_CACHE is None:
        _NC_CACHE = build_nc()
    return _NC_CACHE


def make_in_maps(x, wq, wk, wv, wo):
    import ml_dtypes
    bf16 = ml_dtypes.bfloat16
    xt = np.ascontiguousarray(x.reshape(T, H).T).astype(bf16)
    ones = np.ones((128, 128), dtype=bf16)
    in_maps = []
    for c in range(NCORES):
        qsl = slice(c * G * D, (c + 1) * G * D)
        ksl = slice(c * D, (c + 1) * D)
        in_maps.append({
            "xt": xt,
            "wqt": np.ascontiguousarray(wq[qsl, :].T).astype(bf16),
            "wkt": np.ascontiguousarray(wk[ksl, :].T).astype(bf16),
            "wvt": np.ascontiguousarray(wv[ksl, :].T).astype(bf16),
            "wot": np.ascontiguousarray(wo[:, qsl].T).astype(bf16),
            "ones": ones,
        })
    return in_maps


def kernel(x, wq, wk, wv, wo, **run_kwargs):
    nc = _get_nc()
    in_maps = make_in_maps(np.asarray(x, dtype=np.float32),
                           np.asarray(wq, dtype=np.float32),
                           np.asarray(wk, dtype=np.float32),
                           np.asarray(wv, dtype=np.float32),
                           np.asarray(wo, dtype=np.float32))
    res = run_bass_kernel_spmd(nc, in_maps, core_ids=list(range(NCORES)),
                               **run_kwargs)
    acc = np.zeros((T, H), dtype=np.float32)
    for c in range(NCORES):
        acc += res.results[c]["out"].astype(np.float32)
    out = acc.reshape(B, S, H)
    if run_kwargs:
        return out, res
    return out


# revision 17
# speedup vs baseline: 1.0306x; 1.0306x over previous
"""Trainium2 Bass kernel for Llama GQA attention (no mask), 8-way tensor
parallel over KV heads.

Problem shapes (hardcoded):
  x  (2, 2048, 4096) f32
  wq (4096, 4096), wk (1024, 4096), wv (1024, 4096), wo (4096, 4096) f32
  NUM_HEADS=32, NUM_KV_HEADS=8, HEAD_DIM=128, GQA group g=4

Sharding: core c owns KV head c (4 Q heads). x replicated (pre-transposed
to xT on host), wq/wk/wv sharded on output dim (pre-transposed host-side),
wo sharded on input dim. Each core computes a partial (4096, 4096) output
(its heads' contribution through wo); host sums the 8 partials.

All matmuls run in fp32r (full-rate fp32, HIGH mode single pass).

v1 changes vs baseline (1069us):
  - softmax denominator no longer computed with 512 ones-matmuls on the PE
    (was ~124us of PE busy). Instead the exp chunks are summed on the DVE
    (tensor_add chain into acc, then a 1024->512 fold), and a single
    ones-matmul per (b,j,m) partition-reduces + broadcasts the result into
    PSUM for the reciprocal.
  - the repl-matmul/reciprocal/normalize chain for head m is delayed into
    head m+1's g-loop (slot g=2) so the PE never waits on the DVE sum.
  - output projection groups (4 accumulating MMs each) are interleaved into
    the attention g-loop (slots 4..31, one group per g-step) instead of
    running as a single block: the PE has filler work whenever exp lags,
    and the output DMA is spread across the whole chunk.
  - outproj PSUM evacuation moved from ACT to DVE so ACT only does exp.
  - first weight/x DMA chunks split across partition halves and more queues
    to cut the startup head (~13us -> target ~7us).
"""

import sys
from contextlib import ExitStack

import numpy as np

sys.path.insert(0, "/opt/trn_rl_repo")

import concourse.bass as bass  # noqa: E402
import concourse.tile as tile  # noqa: E402
from concourse import bacc, mybir  # noqa: E402
from concourse.bass_utils import run_bass_kernel_spmd  # noqa: E402
from concourse.masks import make_identity  # noqa: E402

NCORES = 8
B, S, H = 2, 2048, 4096
T = B * S                      # 4096 flattened tokens
D = 128                        # head dim
G = 4                          # q heads per core (GQA group)
HK = 32                        # h k-tiles (4096 / 128)
TT = T // 128                  # 32 token tiles
NJ = T // 512                  # 8 token chunks of 512
SJ = S // 512                  # 4 tq chunks per batch
SI = S // 128                  # 16 tk tiles per batch
SCALE = float(1.0 / np.sqrt(D))

F32 = mybir.dt.float32
F32R = mybir.dt.float32r
BF16 = mybir.dt.bfloat16
COPY = mybir.ActivationFunctionType.Copy
EXP = mybir.ActivationFunctionType.Exp


def build_nc():
    nc = bacc.Bacc("TRN2", target_bir_lowering=False, debug=False,
                   enable_asserts=True, num_devices=NCORES)
    xt = nc.declare_dram_parameter("xt", [H, T], BF16, isOutput=False)
    wqt = nc.declare_dram_parameter("wqt", [H, G * D], BF16, isOutput=False)
    wkt = nc.declare_dram_parameter("wkt", [H, D], BF16, isOutput=False)
    wvt = nc.declare_dram_parameter("wvt", [H, D], BF16, isOutput=False)
    wot = nc.declare_dram_parameter("wot", [G * D, H], BF16, isOutput=False)
    ones = nc.declare_dram_parameter("ones", [128, 128], BF16, isOutput=False)
    out = nc.declare_dram_parameter("out", [T, H], BF16, isOutput=True)

    xt_r = xt.ap().rearrange("(k p) t -> p k t", p=128)     # [128, 32, T]
    wqt_r = wqt.ap().rearrange("(k p) m -> p k m", p=128)   # [128, 32, 512]
    wkt_r = wkt.ap().rearrange("(k p) m -> p k m", p=128)   # [128, 32, 128]
    wvt_r = wvt.ap().rearrange("(k p) m -> p k m", p=128)   # [128, 32, 128]
    wot_r = wot.ap().rearrange("(k p) n -> p k n", p=128)   # [128, 4, T]
    out_r = out.ap()

    with tile.TileContext(nc) as tc:
        with ExitStack() as ctx:
            persist = ctx.enter_context(tc.tile_pool(name="persist", bufs=1))
            q_sb = persist.tile([128, G, T], BF16)       # qT per head, 8MB
            k_sb = persist.tile([128, T], BF16)          # kT, 2MB
            v_sb = persist.tile([128, TT, D], BF16)      # v natural, 2MB
            ones_sb = persist.tile([128, 128], BF16)

            # ---------------- phase 1: projections ----------------
            with ExitStack() as c1:
                wpool = c1.enter_context(tc.tile_pool(name="wpool", bufs=1))
                xpool = c1.enter_context(tc.tile_pool(name="xpool", bufs=6))
                vstg = c1.enter_context(tc.tile_pool(name="vstg", bufs=2))
                ps1 = c1.enter_context(tc.tile_pool(name="ps1", bufs=1, space="PSUM"))
                pstr = c1.enter_context(tc.tile_pool(name="pstr", bufs=2, space="PSUM"))

                wq_t = wpool.tile([128, HK, G * D], BF16)   # 4MB
                wk_t = wpool.tile([128, HK, D], BF16)       # 1MB
                wv_t = wpool.tile([128, HK, D], BF16)       # 1MB
                ident = wpool.tile([128, 128], BF16)
                # first k-tiles land fast (fine splits across two queues);
                # the bulk is batched into a few big DMAs because each gpsimd
                # (SWDGE) dma_start costs ~0.6us of serial Q7 descriptor
                # generation - 100 small triggers would starve the late
                # k-tiles by ~10us
                for k in range(4):
                    if k < 2:
                        for q8 in range(8):
                            eng = [nc.gpsimd, nc.scalar][q8 % 2]
                            eng.dma_start(
                                out=wq_t[:, k, q8 * 64:(q8 + 1) * 64],
                                in_=wqt_r[:, k, q8 * 64:(q8 + 1) * 64])
                        for q2 in range(2):
                            psl = slice(q2 * 64, (q2 + 1) * 64)
                            nc.gpsimd.dma_start(out=wk_t[psl, k, :],
                                                in_=wkt_r[psl, k, :])
                            nc.scalar.dma_start(out=wv_t[psl, k, :],
                                                in_=wvt_r[psl, k, :])
                    else:
                        nc.gpsimd.dma_start(out=wq_t[:, k, :], in_=wqt_r[:, k, :])
                        nc.gpsimd.dma_start(out=wk_t[:, k, :], in_=wkt_r[:, k, :])
                        nc.gpsimd.dma_start(out=wv_t[:, k, :], in_=wvt_r[:, k, :])
                for k0 in range(4, HK, 7):
                    k1 = min(k0 + 7, HK)
                    nc.gpsimd.dma_start(out=wq_t[:, k0:k1, :],
                                        in_=wqt_r[:, k0:k1, :])
                nc.gpsimd.dma_start(out=wk_t[:, 4:HK, :], in_=wkt_r[:, 4:HK, :])
                nc.gpsimd.dma_start(out=wv_t[:, 4:HK, :], in_=wvt_r[:, 4:HK, :])
                make_identity(nc, ident)

                def v_transpose(pj, pv_st):
                    # one-j-delayed so PE never waits on the DVE staging copy
                    vt_ps = pstr.tile([128, 4, 128], BF16)
                    for tt in range(4):
                        nc.tensor.transpose(
                            vt_ps[:, tt, :], pv_st[:, tt * 128:(tt + 1) * 128],
                            ident)
                    nc.scalar.activation(
                        out=v_sb[:, 4 * pj:4 * pj + 4, :], in_=vt_ps, func=COPY)

                prev_v = None
                for j in range(NJ):
                    tsl = slice(j * 512, (j + 1) * 512)
                    q_ps = [ps1.tile([128, 512], F32, name=f"q_ps{m}")
                            for m in range(G)]
                    k_ps = ps1.tile([128, 512], F32)
                    v_ps = ps1.tile([128, 512], F32)
                    for k in range(HK):
                        if k == 0 and prev_v is not None:
                            v_transpose(*prev_v)
                            prev_v = None
                        x_t = xpool.tile([128, 512], BF16)
                        if j == 0 and k < 4:
                            # split first x tiles across partition quarters
                            # to cut their arrival latency
                            for q4 in range(4):
                                psl = slice(q4 * 32, (q4 + 1) * 32)
                                nc.sync.dma_start(out=x_t[psl, :],
                                                  in_=xt_r[psl, k, tsl])
                        else:
                            nc.sync.dma_start(out=x_t, in_=xt_r[:, k, tsl])
                        st = k == 0
                        sp = k == HK - 1
                        for m in range(G):
                            nc.tensor.matmul(
                                q_ps[m], wq_t[:, k, m * D:(m + 1) * D], x_t,
                                start=st, stop=sp)
                        nc.tensor.matmul(k_ps, wk_t[:, k, :], x_t, start=st, stop=sp)
                        nc.tensor.matmul(v_ps, wv_t[:, k, :], x_t, start=st, stop=sp)
                    # split psum evacuation across ACT and DVE so the banks
                    # free up fast for the next j iteration
                    nc.scalar.activation(out=q_sb[:, 0, tsl], in_=q_ps[0], func=COPY)
                    nc.vector.tensor_copy(q_sb[:, 1, tsl], q_ps[1])
                    nc.scalar.activation(out=q_sb[:, 2, tsl], in_=q_ps[2], func=COPY)
                    nc.vector.tensor_copy(q_sb[:, 3, tsl], q_ps[3])
                    nc.scalar.activation(out=k_sb[:, tsl], in_=k_ps, func=COPY)
                    # v: vT [dv, t] -> transpose 128-col blocks -> v [t, dv]
                    v_st = vstg.tile([128, 512], BF16)
                    nc.vector.tensor_copy(v_st, v_ps)
                    prev_v = (j, v_st)
                v_transpose(*prev_v)

            # ------- phase 2: fused attention + output projection -------
            with ExitStack() as c2:
                wopool = c2.enter_context(tc.tile_pool(name="wopool", bufs=1))
                apool = c2.enter_context(tc.tile_pool(name="apool", bufs=2))
                ppool = c2.enter_context(tc.tile_pool(name="ppool", bufs=4))
                accpool = c2.enter_context(tc.tile_pool(name="accpool", bufs=1))
                dpool = c2.enter_context(tc.tile_pool(name="dpool", bufs=2))
                rpool = c2.enter_context(tc.tile_pool(name="rpool", bufs=2))
                opool = c2.enter_context(tc.tile_pool(name="opool", bufs=3))
                psS = c2.enter_context(tc.tile_pool(name="psS", bufs=2, space="PSUM"))
                psPV = c2.enter_context(tc.tile_pool(name="psPV", bufs=2, space="PSUM"))
                psO = c2.enter_context(tc.tile_pool(name="psO", bufs=2, space="PSUM"))

                wo_sb = wopool.tile([128, G, T], BF16)      # 4MB resident
                nc.gpsimd.dma_start(out=ones_sb, in_=ones.ap())
                for k in range(G):
                    nc.gpsimd.dma_start(out=wo_sb[:, k, :], in_=wot_r[:, k, :])

                # one outproj group: 4 accumulating MMs -> [tq 128, h 512]
                # PSUM, evac on DVE, DMA out
                def outproj_group(pb, pj, pa, grp, drain=False):
                    tt2, n = grp // NJ, grp % NJ
                    t0 = pb * S + pj * 512 + tt2 * 128
                    o_ps = psO.tile([128, 512], F32, name="o_ps")
                    for m in range(G):
                        nc.tensor.matmul(
                            o_ps, pa[m][:, tt2 * 128:(tt2 + 1) * 128],
                            wo_sb[:, m, n * 512:(n + 1) * 512],
                            start=(m == 0), stop=(m == G - 1))
                    o_t = opool.tile([128, 512], BF16)
                    # alternate the PSUM evacuation between DVE and ACT so
                    # neither engine paces the o_ps bank rotation (drain: the
                    # exp stream is done, DVE handles all of it)
                    if drain or grp % 2 == 0:
                        nc.vector.tensor_copy(o_t, o_ps)
                    else:
                        nc.scalar.activation(out=o_t, in_=o_ps, func=COPY)
                    nc.sync.dma_start(
                        out=out_r[t0:t0 + 128, n * 512:(n + 1) * 512],
                        in_=o_t)

                # finalize head m: partition-reduce+broadcast den1 via a
                # ones-matmul, reciprocal, normalize pv -> a_ch
                def flush_pending(pend):
                    pv_ps, den1, a_t = pend
                    den_ps = psO.tile([128, 512], F32, name="o_ps")
                    nc.tensor.matmul(den_ps, ones_sb, den1, start=True, stop=True)
                    rec_t = rpool.tile([128, 512], F32)
                    nc.vector.reciprocal_approx_fast(out=rec_t, in_=den_ps)
                    nc.vector.tensor_mul(a_t, pv_ps, rec_t)

                # one PV step (2 accumulating MMs) + the den-add for an exp
                # chunk produced one slot earlier: the one-slot delay keeps
                # the PE from ever waiting on the exp activation. On the
                # head's last chunk it also folds the accumulated exp sums
                # into den1 and returns the (pv, den1, a_ch) finalize record.
                def pv_step(pd):
                    pv_ps, p_t, bb, g, acc, den1, a_t = pd
                    for h in range(2):
                        ti = bb * SI + 2 * g + h
                        st = g == 0 and h == 0
                        sp = g == SI // 2 - 1 and h == 1
                        nc.tensor.matmul(
                            pv_ps, v_sb[:, ti, :],
                            p_t[:, h * 512:(h + 1) * 512],
                            start=st, stop=sp)
                    if g == 0:
                        nc.vector.tensor_copy(acc, p_t)
                    else:
                        nc.vector.tensor_add(acc, acc, p_t)
                    if g == SI // 2 - 1:
                        nc.vector.tensor_add(den1, acc[:, 0:512],
                                             acc[:, 512:1024])
                        return (pv_ps, den1, a_t)
                    return None

                pending = None   # (pv_ps, den1, a_ch target) of previous head
                pvq = []         # exp chunks awaiting their PV matmuls
                prev = None      # (b, j, a_ch list) of previous chunk
                for b in range(B):
                    for j in range(SJ):
                        tqsl = slice(b * S + j * 512, b * S + (j + 1) * 512)
                        a_ch = [apool.tile([128, 512], BF16, name=f"a_ch{m}")
                                for m in range(G)]
                        for m in range(G):
                            pv_ps = psPV.tile([128, 512], F32, name="pv_ps")
                            acc = accpool.tile([128, 1024], F32)
                            den1 = dpool.tile([128, 512], BF16)
                            for g in range(SI // 2):
                                slot = m * (SI // 2) + g
                                s_ps = psS.tile([128, 1024], F32)
                                for h in range(2):
                                    ti = b * SI + 2 * g + h
                                    nc.tensor.matmul(
                                        s_ps[:, h * 512:(h + 1) * 512],
                                        k_sb[:, ti * 128:(ti + 1) * 128],
                                        q_sb[:, m, tqsl], start=True, stop=True)
                                p_t = ppool.tile([128, 1024], BF16)
                                nc.scalar.activation(out=p_t, in_=s_ps, func=EXP,
                                                     scale=SCALE)
                                # interleaved outproj of the previous chunk:
                                # base groups at slots 6..31, the remaining 6
                                # doubled into slots 26..31
                                if prev is not None and slot >= 6:
                                    outproj_group(prev[0], prev[1], prev[2],
                                                  slot - 6)
                                    if slot >= 26:
                                        outproj_group(prev[0], prev[1], prev[2],
                                                      slot)
                                # PV of the exp chunk from two slots ago
                                if len(pvq) >= 2:
                                    fin = pv_step(pvq.pop(0))
                                    if fin is not None:
                                        pending = fin
                                pvq.append((pv_ps, p_t, b, g, acc, den1,
                                            a_ch[m]))
                                # delayed finalize of the previous head
                                if g == 3 and pending is not None:
                                    flush_pending(pending)
                                    pending = None
                        prev = (b, j, a_ch)
                # drain: last PVs, last head, last chunk's outproj
                while pvq:
                    fin = pv_step(pvq.pop(0))
                    if fin is not None:
                        pending = fin
                flush_pending(pending)
                for grp in range(32):
                    outproj_group(prev[0], prev[1], prev[2], grp, drain=True)
    nc.compile()
    return nc


_NC_CACHE = None


def _get_nc():
    global _NC_CACHE
    if _NC_CACHE is None:
        _NC_CACHE = build_nc()
    return _NC_CACHE


def make_in_maps(x, wq, wk, wv, wo):
    import ml_dtypes
    bf16 = ml_dtypes.bfloat16
    xt = np.ascontiguousarray(x.reshape(T, H).T).astype(bf16)
    ones = np.ones((128, 128), dtype=bf16)
    in_maps = []
    for c in range(NCORES):
        qsl = slice(c * G * D, (c + 1) * G * D)
        ksl = slice(c * D, (c + 1) * D)
        in_maps.append({
            "xt": xt,
            "wqt": np.ascontiguousarray(wq[qsl, :].T).astype(bf16),
            "wkt": np.ascontiguousarray(wk[ksl, :].T).astype(bf16),
            "wvt": np.ascontiguousarray(wv[ksl, :].T).astype(bf16),
            "wot": np.ascontiguousarray(wo[:, qsl].T).astype(bf16),
            "ones": ones,
        })
    return in_maps


def kernel(x, wq, wk, wv, wo, **run_kwargs):
    nc = _get_nc()
    in_maps = make_in_maps(np.asarray(x, dtype=np.float32),
                           np.asarray(wq, dtype=np.float32),
                           np.asarray(wk, dtype=np.float32),
                           np.asarray(wv, dtype=np.float32),
                           np.asarray(wo, dtype=np.float32))
    res = run_bass_kernel_spmd(nc, in_maps, core_ids=list(range(NCORES)),
                               **run_kwargs)
    acc = np.zeros((T, H), dtype=np.float32)
    for c in range(NCORES):
        acc += res.results[c]["out"].astype(np.float32)
    out = acc.reshape(B, S, H)
    if run_kwargs:
        return out, res
    return out


# revision 18
# speedup vs baseline: 1.0627x; 1.0312x over previous
"""Trainium2 Bass kernel for Llama GQA attention (no mask), 8-way tensor
parallel over KV heads.

Problem shapes (hardcoded):
  x  (2, 2048, 4096) f32
  wq (4096, 4096), wk (1024, 4096), wv (1024, 4096), wo (4096, 4096) f32
  NUM_HEADS=32, NUM_KV_HEADS=8, HEAD_DIM=128, GQA group g=4

Sharding: core c owns KV head c (4 Q heads). x replicated (pre-transposed
to xT on host), wq/wk/wv sharded on output dim (pre-transposed host-side),
wo sharded on input dim. Each core computes a partial (4096, 4096) output
(its heads' contribution through wo); host sums the 8 partials.

All matmuls run in fp32r (full-rate fp32, HIGH mode single pass).

v1 changes vs baseline (1069us):
  - softmax denominator no longer computed with 512 ones-matmuls on the PE
    (was ~124us of PE busy). Instead the exp chunks are summed on the DVE
    (tensor_add chain into acc, then a 1024->512 fold), and a single
    ones-matmul per (b,j,m) partition-reduces + broadcasts the result into
    PSUM for the reciprocal.
  - the repl-matmul/reciprocal/normalize chain for head m is delayed into
    head m+1's g-loop (slot g=2) so the PE never waits on the DVE sum.
  - output projection groups (4 accumulating MMs each) are interleaved into
    the attention g-loop (slots 4..31, one group per g-step) instead of
    running as a single block: the PE has filler work whenever exp lags,
    and the output DMA is spread across the whole chunk.
  - outproj PSUM evacuation moved from ACT to DVE so ACT only does exp.
  - first weight/x DMA chunks split across partition halves and more queues
    to cut the startup head (~13us -> target ~7us).
"""

import sys
from contextlib import ExitStack

import numpy as np

sys.path.insert(0, "/opt/trn_rl_repo")

import concourse.bass as bass  # noqa: E402
import concourse.tile as tile  # noqa: E402
from concourse import bacc, mybir  # noqa: E402
from concourse.bass_utils import run_bass_kernel_spmd  # noqa: E402
from concourse.masks import make_identity  # noqa: E402

NCORES = 8
B, S, H = 2, 2048, 4096
T = B * S                      # 4096 flattened tokens
D = 128                        # head dim
G = 4                          # q heads per core (GQA group)
HK = 32                        # h k-tiles (4096 / 128)
TT = T // 128                  # 32 token tiles
NJ = T // 512                  # 8 token chunks of 512
SJ = S // 512                  # 4 tq chunks per batch
SI = S // 128                  # 16 tk tiles per batch
SCALE = float(1.0 / np.sqrt(D))

F32 = mybir.dt.float32
F32R = mybir.dt.float32r
BF16 = mybir.dt.bfloat16
COPY = mybir.ActivationFunctionType.Copy
EXP = mybir.ActivationFunctionType.Exp


def build_nc():
    nc = bacc.Bacc("TRN2", target_bir_lowering=False, debug=False,
                   enable_asserts=True, num_devices=NCORES)
    xt = nc.declare_dram_parameter("xt", [H, T], BF16, isOutput=False)
    wqt = nc.declare_dram_parameter("wqt", [H, G * D], BF16, isOutput=False)
    wkt = nc.declare_dram_parameter("wkt", [H, D], BF16, isOutput=False)
    wvt = nc.declare_dram_parameter("wvt", [H, D], BF16, isOutput=False)
    wot = nc.declare_dram_parameter("wot", [G * D, H], BF16, isOutput=False)
    ones = nc.declare_dram_parameter("ones", [128, 128], BF16, isOutput=False)
    out = nc.declare_dram_parameter("out", [T, H], BF16, isOutput=True)

    xt_r = xt.ap().rearrange("(k p) t -> p k t", p=128)     # [128, 32, T]
    wqt_r = wqt.ap().rearrange("(k p) m -> p k m", p=128)   # [128, 32, 512]
    wkt_r = wkt.ap().rearrange("(k p) m -> p k m", p=128)   # [128, 32, 128]
    wvt_r = wvt.ap().rearrange("(k p) m -> p k m", p=128)   # [128, 32, 128]
    wot_r = wot.ap().rearrange("(k p) n -> p k n", p=128)   # [128, 4, T]
    out_r = out.ap()

    with tile.TileContext(nc) as tc:
        with ExitStack() as ctx:
            persist = ctx.enter_context(tc.tile_pool(name="persist", bufs=1))
            q_sb = persist.tile([128, G, T], BF16)       # qT per head, 8MB
            k_sb = persist.tile([128, T], BF16)          # kT, 2MB
            v_sb = persist.tile([128, TT, D], BF16)      # v natural, 2MB
            ones_sb = persist.tile([128, 128], BF16)

            # ---------------- phase 1: projections ----------------
            with ExitStack() as c1:
                wpool = c1.enter_context(tc.tile_pool(name="wpool", bufs=1))
                xpool = c1.enter_context(tc.tile_pool(name="xpool", bufs=6))
                vstg = c1.enter_context(tc.tile_pool(name="vstg", bufs=2))
                ps1 = c1.enter_context(tc.tile_pool(name="ps1", bufs=1, space="PSUM"))
                pstr = c1.enter_context(tc.tile_pool(name="pstr", bufs=2, space="PSUM"))

                wq_t = wpool.tile([128, HK, G * D], BF16)   # 4MB
                wk_t = wpool.tile([128, HK, D], BF16)       # 1MB
                wv_t = wpool.tile([128, HK, D], BF16)       # 1MB
                ident = wpool.tile([128, 128], BF16)
                # first k-tiles land fast (fine splits across two queues);
                # the bulk is batched into a few big DMAs because each gpsimd
                # (SWDGE) dma_start costs ~0.6us of serial Q7 descriptor
                # generation - 100 small triggers would starve the late
                # k-tiles by ~10us
                for k in range(4):
                    if k < 2:
                        for q8 in range(8):
                            eng = [nc.gpsimd, nc.scalar][q8 % 2]
                            eng.dma_start(
                                out=wq_t[:, k, q8 * 64:(q8 + 1) * 64],
                                in_=wqt_r[:, k, q8 * 64:(q8 + 1) * 64])
                        for q2 in range(2):
                            psl = slice(q2 * 64, (q2 + 1) * 64)
                            nc.gpsimd.dma_start(out=wk_t[psl, k, :],
                                                in_=wkt_r[psl, k, :])
                            nc.scalar.dma_start(out=wv_t[psl, k, :],
                                                in_=wvt_r[psl, k, :])
                    else:
                        nc.gpsimd.dma_start(out=wq_t[:, k, :], in_=wqt_r[:, k, :])
                        nc.gpsimd.dma_start(out=wk_t[:, k, :], in_=wkt_r[:, k, :])
                        nc.gpsimd.dma_start(out=wv_t[:, k, :], in_=wvt_r[:, k, :])
                for k0 in range(4, HK, 7):
                    k1 = min(k0 + 7, HK)
                    nc.gpsimd.dma_start(out=wq_t[:, k0:k1, :],
                                        in_=wqt_r[:, k0:k1, :])
                nc.gpsimd.dma_start(out=wk_t[:, 4:HK, :], in_=wkt_r[:, 4:HK, :])
                nc.gpsimd.dma_start(out=wv_t[:, 4:HK, :], in_=wvt_r[:, 4:HK, :])
                make_identity(nc, ident)

                def v_transpose(pj, pv_st):
                    # one-j-delayed so PE never waits on the DVE staging copy
                    vt_ps = pstr.tile([128, 4, 128], BF16)
                    for tt in range(4):
                        nc.tensor.transpose(
                            vt_ps[:, tt, :], pv_st[:, tt * 128:(tt + 1) * 128],
                            ident)
                    nc.scalar.activation(
                        out=v_sb[:, 4 * pj:4 * pj + 4, :], in_=vt_ps, func=COPY)

                prev_v = None
                for j in range(NJ):
                    tsl = slice(j * 512, (j + 1) * 512)
                    q_ps = [ps1.tile([128, 512], F32, name=f"q_ps{m}")
                            for m in range(G)]
                    k_ps = ps1.tile([128, 512], F32)
                    v_ps = ps1.tile([128, 512], F32)
                    x4s = {}
                    for k in range(HK):
                        if k == 0 and prev_v is not None:
                            v_transpose(*prev_v)
                            prev_v = None
                        if j == 0 and k >= 4:
                            # ramp: batch 4 k-tiles per DMA, alternating
                            # rings, so supply stays ahead of the PE
                            k4 = k // 4
                            if k % 4 == 0:
                                x4s[k4] = xpool.tile([128, 4, 512], BF16,
                                                     name="x4")
                                ring = nc.sync if k4 % 2 == 0 else nc.scalar
                                ring.dma_start(
                                    out=x4s[k4],
                                    in_=xt_r[:, 4 * k4:4 * k4 + 4, tsl])
                            x_t = x4s[k4][:, k % 4, :]
                        elif j == 0:
                            x_t = xpool.tile([128, 512], BF16)
                            # split first x tiles across partition quarters
                            # to cut their arrival latency
                            for q4 in range(4):
                                psl = slice(q4 * 32, (q4 + 1) * 32)
                                nc.sync.dma_start(out=x_t[psl, :],
                                                  in_=xt_r[psl, k, tsl])
                        else:
                            x_t = xpool.tile([128, 512], BF16)
                            nc.sync.dma_start(out=x_t, in_=xt_r[:, k, tsl])
                        st = k == 0
                        sp = k == HK - 1
                        for m in range(G):
                            nc.tensor.matmul(
                                q_ps[m], wq_t[:, k, m * D:(m + 1) * D], x_t,
                                start=st, stop=sp)
                        nc.tensor.matmul(k_ps, wk_t[:, k, :], x_t, start=st, stop=sp)
                        nc.tensor.matmul(v_ps, wv_t[:, k, :], x_t, start=st, stop=sp)
                    # split psum evacuation across ACT and DVE so the banks
                    # free up fast for the next j iteration
                    nc.scalar.activation(out=q_sb[:, 0, tsl], in_=q_ps[0], func=COPY)
                    nc.vector.tensor_copy(q_sb[:, 1, tsl], q_ps[1])
                    nc.scalar.activation(out=q_sb[:, 2, tsl], in_=q_ps[2], func=COPY)
                    nc.vector.tensor_copy(q_sb[:, 3, tsl], q_ps[3])
                    nc.scalar.activation(out=k_sb[:, tsl], in_=k_ps, func=COPY)
                    # v: vT [dv, t] -> transpose 128-col blocks -> v [t, dv]
                    v_st = vstg.tile([128, 512], BF16)
                    nc.vector.tensor_copy(v_st, v_ps)
                    prev_v = (j, v_st)
                v_transpose(*prev_v)

            # ------- phase 2: fused attention + output projection -------
            with ExitStack() as c2:
                wopool = c2.enter_context(tc.tile_pool(name="wopool", bufs=1))
                apool = c2.enter_context(tc.tile_pool(name="apool", bufs=2))
                ppool = c2.enter_context(tc.tile_pool(name="ppool", bufs=4))
                accpool = c2.enter_context(tc.tile_pool(name="accpool", bufs=1))
                dpool = c2.enter_context(tc.tile_pool(name="dpool", bufs=2))
                rpool = c2.enter_context(tc.tile_pool(name="rpool", bufs=2))
                opool = c2.enter_context(tc.tile_pool(name="opool", bufs=3))
                psS = c2.enter_context(tc.tile_pool(name="psS", bufs=2, space="PSUM"))
                psPV = c2.enter_context(tc.tile_pool(name="psPV", bufs=2, space="PSUM"))
                psO = c2.enter_context(tc.tile_pool(name="psO", bufs=2, space="PSUM"))

                wo_sb = wopool.tile([128, G, T], BF16)      # 4MB resident
                nc.gpsimd.dma_start(out=ones_sb, in_=ones.ap())
                for k in range(G):
                    nc.gpsimd.dma_start(out=wo_sb[:, k, :], in_=wot_r[:, k, :])

                # one outproj group: 4 accumulating MMs -> [tq 128, h 512]
                # PSUM, evac on DVE, DMA out
                def outproj_group(pb, pj, pa, grp, drain=False):
                    tt2, n = grp // NJ, grp % NJ
                    t0 = pb * S + pj * 512 + tt2 * 128
                    o_ps = psO.tile([128, 512], F32, name="o_ps")
                    for m in range(G):
                        nc.tensor.matmul(
                            o_ps, pa[m][:, tt2 * 128:(tt2 + 1) * 128],
                            wo_sb[:, m, n * 512:(n + 1) * 512],
                            start=(m == 0), stop=(m == G - 1))
                    o_t = opool.tile([128, 512], BF16)
                    # alternate the PSUM evacuation between DVE and ACT so
                    # neither engine paces the o_ps bank rotation (drain: the
                    # exp stream is done, DVE handles all of it)
                    if drain or grp % 2 == 0:
                        nc.vector.tensor_copy(o_t, o_ps)
                    else:
                        nc.scalar.activation(out=o_t, in_=o_ps, func=COPY)
                    nc.sync.dma_start(
                        out=out_r[t0:t0 + 128, n * 512:(n + 1) * 512],
                        in_=o_t)

                # finalize head m: partition-reduce+broadcast den1 via a
                # ones-matmul, reciprocal, normalize pv -> a_ch
                def flush_pending(pend):
                    pv_ps, den1, a_t = pend
                    den_ps = psO.tile([128, 512], F32, name="o_ps")
                    nc.tensor.matmul(den_ps, ones_sb, den1, start=True, stop=True)
                    rec_t = rpool.tile([128, 512], F32)
                    nc.vector.reciprocal_approx_fast(out=rec_t, in_=den_ps)
                    nc.vector.tensor_mul(a_t, pv_ps, rec_t)

                # one PV step (2 accumulating MMs) + the den-add for an exp
                # chunk produced one slot earlier: the one-slot delay keeps
                # the PE from ever waiting on the exp activation. On the
                # head's last chunk it also folds the accumulated exp sums
                # into den1 and returns the (pv, den1, a_ch) finalize record.
                def pv_step(pd):
                    pv_ps, p_t, bb, g, acc, den1, a_t = pd
                    for h in range(2):
                        ti = bb * SI + 2 * g + h
                        st = g == 0 and h == 0
                        sp = g == SI // 2 - 1 and h == 1
                        nc.tensor.matmul(
                            pv_ps, v_sb[:, ti, :],
                            p_t[:, h * 512:(h + 1) * 512],
                            start=st, stop=sp)
                    if g == 0:
                        nc.vector.tensor_copy(acc, p_t)
                    else:
                        nc.vector.tensor_add(acc, acc, p_t)
                    if g == SI // 2 - 1:
                        nc.vector.tensor_add(den1, acc[:, 0:512],
                                             acc[:, 512:1024])
                        return (pv_ps, den1, a_t)
                    return None

                pending = None   # (pv_ps, den1, a_ch target) of previous head
                pvq = []         # exp chunks awaiting their PV matmuls
                prev = None      # (b, j, a_ch list) of previous chunk
                for b in range(B):
                    for j in range(SJ):
                        tqsl = slice(b * S + j * 512, b * S + (j + 1) * 512)
                        a_ch = [apool.tile([128, 512], BF16, name=f"a_ch{m}")
                                for m in range(G)]
                        for m in range(G):
                            pv_ps = psPV.tile([128, 512], F32, name="pv_ps")
                            acc = accpool.tile([128, 1024], BF16)
                            den1 = dpool.tile([128, 512], BF16)
                            for g in range(SI // 2):
                                slot = m * (SI // 2) + g
                                s_ps = psS.tile([128, 1024], F32)
                                for h in range(2):
                                    ti = b * SI + 2 * g + h
                                    nc.tensor.matmul(
                                        s_ps[:, h * 512:(h + 1) * 512],
                                        k_sb[:, ti * 128:(ti + 1) * 128],
                                        q_sb[:, m, tqsl], start=True, stop=True)
                                p_t = ppool.tile([128, 1024], BF16)
                                nc.scalar.activation(out=p_t, in_=s_ps, func=EXP,
                                                     scale=SCALE)
                                # interleaved outproj of the previous chunk:
                                # base groups at slots 6..31, the remaining 6
                                # doubled into slots 26..31
                                if prev is not None and slot >= 6:
                                    outproj_group(prev[0], prev[1], prev[2],
                                                  slot - 6)
                                    if slot >= 26:
                                        outproj_group(prev[0], prev[1], prev[2],
                                                      slot)
                                # PV of the exp chunk from two slots ago
                                if len(pvq) >= 2:
                                    fin = pv_step(pvq.pop(0))
                                    if fin is not None:
                                        pending = fin
                                pvq.append((pv_ps, p_t, b, g, acc, den1,
                                            a_ch[m]))
                                # delayed finalize of the previous head
                                if g == 3 and pending is not None:
                                    flush_pending(pending)
                                    pending = None
                        prev = (b, j, a_ch)
                # drain: last PVs, last head, last chunk's outproj
                while pvq:
                    fin = pv_step(pvq.pop(0))
                    if fin is not None:
                        pending = fin
                flush_pending(pending)
                for grp in range(32):
                    outproj_group(prev[0], prev[1], prev[2], grp, drain=True)
    nc.compile()
    return nc


_NC_CACHE = None


def _get_nc():
    global _NC_CACHE
    if _NC_CACHE is None:
        _NC_CACHE = build_nc()
    return _NC_CACHE


def make_in_maps(x, wq, wk, wv, wo):
    import ml_dtypes
    bf16 = ml_dtypes.bfloat16
    xt = np.ascontiguousarray(x.reshape(T, H).T).astype(bf16)
    ones = np.ones((128, 128), dtype=bf16)
    in_maps = []
    for c in range(NCORES):
        qsl = slice(c * G * D, (c + 1) * G * D)
        ksl = slice(c * D, (c + 1) * D)
        in_maps.append({
            "xt": xt,
            "wqt": np.ascontiguousarray(wq[qsl, :].T).astype(bf16),
            "wkt": np.ascontiguousarray(wk[ksl, :].T).astype(bf16),
            "wvt": np.ascontiguousarray(wv[ksl, :].T).astype(bf16),
            "wot": np.ascontiguousarray(wo[:, qsl].T).astype(bf16),
            "ones": ones,
        })
    return in_maps


def kernel(x, wq, wk, wv, wo, **run_kwargs):
    nc = _get_nc()
    in_maps = make_in_maps(np.asarray(x, dtype=np.float32),
                           np.asarray(wq, dtype=np.float32),
                           np.asarray(wk, dtype=np.float32),
                           np.asarray(wv, dtype=np.float32),
                           np.asarray(wo, dtype=np.float32))
    res = run_bass_kernel_spmd(nc, in_maps, core_ids=list(range(NCORES)),
                               **run_kwargs)
    acc = np.zeros((T, H), dtype=np.float32)
    for c in range(NCORES):
        acc += res.results[c]["out"].astype(np.float32)
    out = acc.reshape(B, S, H)
    if run_kwargs:
        return out, res
    return out
